# revision 8
# baseline (speedup 1.0000x reference)
"""KANvolution Trainium2 Bass kernel (v5: telescoped ramp basis).

Math: per patch element x and per (f,c,ki,kj):
    K(x) = w_spline * sum_g hat_g(clip(x)) * cp_g  +  w_silu * silu(x)
with hat_g the normalized linear B-spline basis on the 17-knot grid in
[-1,1].  The PWL interpolation of knot values v_k (k = -8..8 in u = 8x
space) telescopes into a RAMP basis:
    PWL(u) = v_{-8} + sum_{k=-8}^{7} (v_{k+1} - v_k) * clamp(u - k, 0, 1)
which (a) needs 16 features per channel instead of 17 hats, (b) handles
the clip() for free (ramps saturate), (c) packs perfectly: 4 k-tiles of
[4 knots x 32 ch] per tap, no pad rows, and (d) folds the v_{-8} sum
into the bias row.  Only one ramp per (c,tap) is fractional (the rest
are exactly 0/1 in bf16), so feature quantization error is tiny.

k-tiles per tap (ki,kj): t=0..3 ramp tiles (128 rows).  Tail: 9 taps x
32 silu rows + 1 bias row = 289 rows packed into 3 k-tiles via
pre-shifted silu planes (SBUF->SBUF DMA copies with the tap's spatial
offset baked in).  Total 39 passes x 4 chunks = 156 matmuls
[K<=128, M=64] x [K, 512] vs the hat version's 180.

PE: F=64 fills half the 128-wide array; passes alternate column groups
(PSUM base partition 0/64) which stream concurrently.  Host sums the
two fp16 output halves.  x arrives host-side pre-replicated to 128
partitions (4 copies of the 32 channels) so features start straight
from SBUF -- no PE replication matmuls.

Features: t0/t1 ramps on DVE (2 chained-ALU tensor_scalar passes),
t2/t3 on ACT (Relu w/ per-partition bias) + DVE min, silu on ACT --
balanced so neither engine gates the ~20us matmul phase.

Sharding: 8 cores = (batch b, output-row half); each core computes
(32, 64, 64) of the output.
"""

import numpy as np
from contextlib import ExitStack

import concourse.bacc as bacc
import concourse.mybir as mybir
import concourse.tile as tile
from concourse.bass_utils import run_bass_kernel_spmd

# Problem constants (hardcoded per harness contract)
B, H, W, C, F = 4, 66, 66, 32, 64
KH = KW = 3
G = 16                                   # spline intervals; G+1 = 17 knots
HO, WO = H - KH + 1, W - KW + 1          # 64, 64
N_CORES = 8
ROWS_PER_CORE = HO // 2                  # 32 output rows
IN_ROWS = ROWS_PER_CORE + KH - 1         # 34 input rows
SPAT = IN_ROWS * W                       # 2244 input spatial positions
SPAT_PAD = 2304                          # pad to 4*576
N_TAPS = KH * KW                         # 9
N_RTILES = 4                             # ramp k-tiles per tap
N_PASS = N_RTILES * N_TAPS + 3           # 36 interior + 3 packed tail
CHUNK_ROWS = 8                           # output rows per matmul chunk
N_CHUNKS = ROWS_PER_CORE // CHUNK_ROWS   # 4
NFREE = CHUNK_ROWS * WO                  # 512 moving-dim per matmul
SLAB = 576
SLABS = [(0, 576), (576, 1152), (1152, 1728), (1728, 2304)]
N_WARMUP = 6                             # clock-ramp junk matmuls (K=128)

_COMPILED = None  # cached (nc) program


def _build_weights(control_points, w_spline, w_silu, bias):
    """[128, 39*64] bf16 weight blocks, one 64-col block per pass.

    Pass p = t*9 + tap (t<4): row r*32+c = dv at ramp knot k=-8+4t+r for
    (tap, c); dv = v_{k+1} - v_k with v = w_spline*cp/(1+1e-8).
    Pass 36: silu taps 0-3; 37: taps 4-7; 38: tap 8 (rows 0-31) +
    bias row 32 = bias + sum_{c,i,j} v_{-8}.
    """
    import ml_dtypes
    cp = control_points.astype(np.float64)
    ws = w_spline.astype(np.float64)
    v = ws[..., None] * cp / (1.0 + 1e-8)          # (F, C, 3, 3, 17)
    dv = v[..., 1:] - v[..., :-1]                  # (F, C, 3, 3, 16)
    wsl = w_silu.astype(np.float64)

    w_all = np.zeros((N_PASS, 128, F), dtype=np.float64)
    for i in range(KH):
        for j in range(KW):
            tap = i * KW + j
            for t in range(N_RTILES):
                for r in range(4):
                    g = 4 * t + r                  # ramp index 0..15
                    w_all[t * N_TAPS + tap, r * 32:(r + 1) * 32, :] = \
                        dv[:, :, i, j, g].T
            m, a = divmod(tap, 4)                  # tail pass 36+m, slot a
            w_all[36 + m, a * 32:(a + 1) * 32, :] = wsl[:, :, i, j].T
    w_all[38, 32, :] = (bias.astype(np.float64)
                        + v[:, :, :, :, 0].sum(axis=(1, 2, 3)))
    w_host = w_all.transpose(1, 0, 2).reshape(128, N_PASS * F)
    return np.ascontiguousarray(w_host.astype(ml_dtypes.bfloat16))


def _build_program():
    nc = bacc.Bacc("TRN2", target_bir_lowering=False, debug=False,
                   num_devices=N_CORES)
    f32 = mybir.dt.float32
    bf16 = mybir.dt.bfloat16
    fp16 = mybir.dt.float16
    AF = mybir.ActivationFunctionType
    OP = mybir.AluOpType
    import os
    # CoreSim has no Silu; swap in Sigmoid for sim-only structure checks.
    AF_SILU = AF.Sigmoid if os.environ.get("KAN_SIM_SAFE") else AF.Silu

    x_in = nc.declare_dram_parameter("x8t", [128, SPAT_PAD], bf16,
                                     isOutput=False)
    w_in = nc.declare_dram_parameter("w", [128, N_PASS * F], bf16,
                                     isOutput=False)
    kv_in = nc.declare_dram_parameter("kv", [128, 8], f32, isOutput=False)
    y_out = nc.declare_dram_parameter("y", [128, N_CHUNKS * NFREE], fp16,
                                      isOutput=True)

    with tile.TileContext(nc) as tc:
        with ExitStack() as ctx:
            sb = ctx.enter_context(tc.tile_pool(name="sb", bufs=1))
            ps = ctx.enter_context(tc.tile_pool(name="ps", bufs=1, space="PSUM"))
            ob = ctx.enter_context(tc.tile_pool(name="ob", bufs=1))

            # --- input DMAs on all three DMA-capable queues, need-order ---
            kv_sb = sb.tile([128, 8], f32, tag="kv")
            nc.sync.dma_start(kv_sb[:], kv_in[:])
            x_sb = sb.tile([128, SPAT_PAD], bf16, tag="xsb")
            w_sb = sb.tile([128, N_PASS * F], bf16, tag="w")
            # x slabs spread over all queues so every slab lands early;
            # weight blocks interleave in pass-need order.
            nc.sync.dma_start(x_sb[:, 0:576], x_in[:, 0:576])
            nc.scalar.dma_start(x_sb[:, 576:1152], x_in[:, 576:1152])
            nc.gpsimd.dma_start(w_sb[:, 0:640], w_in[:, 0:640])      # p0-9
            nc.scalar.dma_start(x_sb[:, 1152:1728], x_in[:, 1152:1728])
            nc.gpsimd.dma_start(x_sb[:, 1728:2304], x_in[:, 1728:2304])
            nc.gpsimd.dma_start(w_sb[:, 640:1472], w_in[:, 640:1472])
            nc.scalar.dma_start(w_sb[:, 1472:2496], w_in[:, 1472:2496])

            # feature planes: 4 ramp tiles + 3 packed silu/bias tail tiles
            ramp = [sb.tile([128, SPAT_PAD], bf16, name=f"rp{t}", tag=f"rp{t}")
                    for t in range(N_RTILES)]
            tp = [sb.tile([128, SPAT_PAD], bf16, name=f"tp{m}", tag=f"tp{m}")
                  for m in range(3)]
            silu_can = sb.tile([32, SPAT_PAD], bf16, tag="silu")
            zt = sb.tile([128, NFREE], bf16, tag="zt")
            tb = [sb.tile([128, SPAT_PAD], bf16, name=f"tb{u}", tag=f"tb{u}")
                  for u in range(2)]

            P = [ps.tile([128, 2 * NFREE], f32, name=f"po{q}", tag=f"po{q}")
                 for q in range(N_CHUNKS)]

            # HAM/clock warm-up junk matmuls; write P[3]'s B region which the
            # real accumulation's start=True later clears.
            nc.vector.memset(zt[:], 0.0)
            nc.vector.memset(tp[2][32:33, :], 1.0)   # bias ones row
            for u in range(N_WARMUP):
                nc.tensor.matmul(P[3][64:128, NFREE:2 * NFREE],
                                 zt[:, 0:F], zt[:],
                                 start=True, stop=True)

            def features(t):
                """Ramp k-tile t: clamp(x8 - k, 0, 1); k in kv[:, t] as -k."""
                for a, b in SLABS:
                    cs = slice(a, b)
                    if t < 2:
                        # pure DVE: (x8 + (-k)) max 0, then min 1 / max 0
                        nc.vector.tensor_scalar(tb[0][:, cs], x_sb[:, cs],
                                                kv_sb[:, t:t + 1], 0.0,
                                                OP.add, OP.max)
                        nc.vector.tensor_scalar(ramp[t][:, cs], tb[0][:, cs],
                                                1.0, 0.0, OP.min, OP.max)
                    else:
                        # ACT Relu(x8 - k) then DVE min 1
                        nc.scalar.activation(tb[1][:, cs], x_sb[:, cs],
                                             AF.Relu,
                                             bias=kv_sb[:, t:t + 1], scale=1.0)
                        nc.vector.tensor_scalar(ramp[t][:, cs], tb[1][:, cs],
                                                1.0, 0.0, OP.min, OP.max)

            started = set()

            def emit_mm(p, qlist, gq=None, stop=False):
                g_ = p % 2 if gq is None else gq
                if p < 36:
                    t, tap = divmod(p, N_TAPS)
                    i, j = divmod(tap, KW)
                    kk, plane = 128, ramp[t]
                else:
                    m = p - 36
                    kk = 128 if m < 2 else 33
                    i = j = 0
                    plane = tp[m]
                col = p * F
                lhsT = w_sb[0:kk, col:col + F]
                for q in qlist:
                    base = (CHUNK_ROWS * q + i) * W
                    rhs = (plane[0:kk, base:base + CHUNK_ROWS * W]
                           .rearrange("p (r w) -> p r w", w=W)
                           [:, :, j:j + WO])
                    nc.tensor.matmul(
                        P[q][F * g_:F * (g_ + 1),
                             NFREE * g_:NFREE * (g_ + 1)]
                            .rearrange("f (r w) -> f r w", w=WO),
                        lhsT, rhs,
                        start=((q, g_) not in started), stop=stop,
                    )
                    started.add((q, g_))

            def emit_out(q, aq=None):
                # PSUM -> SBUF fp16 halves (host sums); ACT copies group A,
                # DVE group B; each half DMAs from its own queue.
                stage = ob.tile([128, NFREE], fp16, tag=f"stage{q}")
                nc.scalar.copy(stage[0:F, :], P[q][0:F, 0:NFREE])
                (aq or nc.sync).dma_start(
                    y_out[0:F, NFREE * q:NFREE * (q + 1)], stage[0:F, :])
                nc.vector.tensor_copy(stage[F:128, :],
                                      P[q][F:128, NFREE:2 * NFREE])
                nc.gpsimd.dma_start(y_out[F:128, NFREE * q:NFREE * (q + 1)],
                                    stage[F:128, :])

            # all features up front in engine-need order (queues are FIFO;
            # matmuls wait on data semaphores, not emission position)
            features(0)                 # DVE
            features(1)                 # DVE
            features(2)                 # ACT Relu + DVE min
            features(3)                 # ACT Relu + DVE min
            # silu canonical plane + pre-shifted copies into tail tiles
            for a, b in SLABS:
                nc.scalar.activation(silu_can[:, a:b], x_sb[0:32, a:b],
                                     AF_SILU, scale=0.125)
            for tap in range(N_TAPS):
                i, j = divmod(tap, KW)
                off = i * W + j
                m, a = divmod(tap, 4)
                eng = (nc.sync, nc.scalar, nc.gpsimd)[tap % 3]
                eng.dma_start(tp[m][a * 32:(a + 1) * 32, 0:SPAT_PAD - off],
                              silu_can[:, off:SPAT_PAD])

            # interior ramp passes
            for p in range(36):
                emit_mm(p, range(N_CHUNKS))
            # packed tail passes: per-chunk group parity keeps both column
            # groups streaming through the close-out; output DMA overlaps
            for q in range(N_CHUNKS):
                emit_mm(36, (q,), gq=q % 2, stop=False)
                emit_mm(37, (q,), gq=(q + 1) % 2, stop=True)
                emit_mm(38, (q,), gq=q % 2, stop=True)
                emit_out(q, aq=nc.scalar if q == 3 else None)

    nc.compile()
    return nc


def _get_program():
    global _COMPILED
    if _COMPILED is None:
        _COMPILED = _build_program()
    return _COMPILED


def _make_in_maps(x, control_points, w_spline, w_silu, bias):
    import ml_dtypes
    bf = ml_dtypes.bfloat16
    w_host = _build_weights(control_points, w_spline, w_silu, bias)

    # kv[:, t] = -k for ramp tile t: k = -8 + 4t + p//32
    kv = np.zeros((128, 8), dtype=np.float32)
    for t in range(N_RTILES):
        for p in range(128):
            kv[p, t] = 8.0 - (4 * t + p // 32)

    x8 = (np.asarray(x, dtype=np.float32) * 8.0).astype(bf)
    in_maps = []
    for core in range(N_CORES):
        b, half = divmod(core, 2)
        r0 = half * ROWS_PER_CORE
        xs = np.zeros((128, SPAT_PAD), dtype=bf)
        flat = x8[b, r0:r0 + IN_ROWS].reshape(SPAT, C).T
        for rep in range(4):
            xs[rep * 32:(rep + 1) * 32, :SPAT] = flat
        in_maps.append({"x8t": xs, "w": w_host, "kv": kv})
    return in_maps


def kernel(x, control_points, w_spline, w_silu, bias):
    in_maps = _make_in_maps(x, control_points, w_spline, w_silu, bias)
    nc = _get_program()
    res = run_bass_kernel_spmd(nc, in_maps, list(range(N_CORES)))

    out = np.empty((B, HO, WO, F), dtype=np.float32)
    for core in range(N_CORES):
        b, half = divmod(core, 2)
        r0 = half * ROWS_PER_CORE
        y2 = res.results[core]["y"].astype(np.float32)   # [128, 2048] fp16
        y = y2[0:F] + y2[F:128]                          # [64, 2048]
        out[b, r0:r0 + ROWS_PER_CORE] = (
            y.reshape(F, ROWS_PER_CORE, WO).transpose(1, 2, 0))
    return out
